# revision 2
# baseline (speedup 1.0000x reference)
"""Self-contained Trainium2 Bass kernel for nn_AttentionHead.

Reference computation (per batch b):
    Q = x @ Wq.T + bq ; K = x @ Wk.T + bk ; V = x @ Wv.T + bv
    scores = Q @ K.T / sqrt(S)          (S = 4096, the reference's seq-len quirk)
    scores = where(mask, -1e9, scores)
    ctx = softmax(scores, -1) @ V

Sharding: 8 cores, each takes one (batch, query-half) pair: core c -> batch
c//2, queries [(c%2)*2048, (c%2+1)*2048). K/V are computed per-core from the
full batch input (cheap, avoids collectives entirely).

Device layout (per core):
  - x passed pre-transposed (xT [D,S] bf16) so all matmuls need no on-device
    transposes: projections contract over d with d on partitions.
  - QT/KT [e, s] (e on partitions), V [s, e] natural.
  - scoresT[k, q] = KT.T-slices @ QT  -> softmax probs PT[k, q] with k on
    partitions; L[q] arrives as an extra ones-column appended to V via a
    rank-1 bias matmul, so ctx_psum[q, 0:256] = P@V and ctx_psum[q, 256] = L.
  - mask applied multiplicatively after exp (exp(-1e9) == 0 in the reference
    for every finite row, and rows cannot be fully masked for random masks).
"""

import sys

sys.path.insert(0, "/opt/trn_rl_repo")

import ml_dtypes
import numpy as np

import concourse.bass as bass
import concourse.tile as tile
from concourse import bacc, mybir
from concourse.bass_utils import run_bass_kernel_spmd

BF16 = ml_dtypes.bfloat16

B, S, D = 4, 4096, 256
NCORES = 8
QC = (B * S) // NCORES  # 2048 queries per core
P = 128

# Mask storage dtype on device: "uint8" or "bfloat16" (fallback if u8 tensor
# ops are unsupported).
MASK_MODE = "uint8"


def build_nc(S_=S, QC_=QC, QW=512, repeats=1, mask_mode=MASK_MODE, debug=False):
    """Build the per-core Bass program (same program runs SPMD on all cores)."""
    D_ = D
    KB = S_ // P            # k blocks of 128
    NW = QC_ // QW          # query windows
    QB = QW // P            # 128-row query blocks per window
    MG = min(8, KB)         # kb per mask-multiply op
    E1 = D_ + 1             # V plus ones column
    f32 = mybir.dt.float32
    bf16 = mybir.dt.bfloat16
    u8 = mybir.dt.uint8
    mdt = u8 if mask_mode == "uint8" else bf16

    nc = bacc.Bacc("TRN2", target_bir_lowering=False, debug=debug,
                   num_devices=NCORES)

    xT = nc.dram_tensor("xT", [D_, S_], bf16, kind="ExternalInput").ap()
    xqT = nc.dram_tensor("xqT", [D_, QC_], bf16, kind="ExternalInput").ap()
    wqT = nc.dram_tensor("wqT", [D_, D_], bf16, kind="ExternalInput").ap()
    wkT = nc.dram_tensor("wkT", [D_, D_], bf16, kind="ExternalInput").ap()
    wvT = nc.dram_tensor("wvT", [D_, E1], bf16, kind="ExternalInput").ap()
    bq2 = nc.dram_tensor("bq2", [P, 2], f32, kind="ExternalInput").ap()
    bk2 = nc.dram_tensor("bk2", [P, 2], f32, kind="ExternalInput").ap()
    bv1 = nc.dram_tensor("bv1", [1, E1], bf16, kind="ExternalInput").ap()
    ones1 = nc.dram_tensor("ones1", [1, P], bf16, kind="ExternalInput").ap()
    validb = nc.dram_tensor("validb", [NW, P, KB * QW], mdt,
                            kind="ExternalInput").ap()
    out = nc.dram_tensor("out", [QC_, D_], f32, kind="ExternalOutput").ap()

    Exp = mybir.ActivationFunctionType.Exp
    mult = mybir.AluOpType.mult

    with tile.TileContext(nc) as tc:
        with (
            tc.tile_pool(name="const", bufs=1) as const,
            tc.tile_pool(name="xt", bufs=1) as xt_pool,
            tc.tile_pool(name="kqv", bufs=1) as kqv_pool,
            tc.tile_pool(name="valid", bufs=2) as valid_pool,
            tc.tile_pool(name="pt", bufs=2) as pt_pool,
            tc.tile_pool(name="ctx", bufs=3) as ctx_pool,
            tc.tile_pool(name="misc", bufs=4) as misc_pool,
            tc.tile_pool(name="ps_s", bufs=2, space="PSUM") as ps_s_pool,
            tc.tile_pool(name="ps_c", bufs=4, space="PSUM") as ps_c_pool,
        ):
            # ---- constants / weights ----
            wq_sb = [const.tile([P, D_], bf16, tag=f"wq{d}", name=f"wq{d}") for d in range(2)]
            wk_sb = [const.tile([P, D_], bf16, tag=f"wk{d}", name=f"wk{d}") for d in range(2)]
            wv_sb = [const.tile([P, E1], bf16, tag=f"wv{d}", name=f"wv{d}") for d in range(2)]
            for d in range(2):
                nc.sync.dma_start(wq_sb[d][:], wqT[d * P:(d + 1) * P, :])
                nc.sync.dma_start(wk_sb[d][:], wkT[d * P:(d + 1) * P, :])
                nc.sync.dma_start(wv_sb[d][:], wvT[d * P:(d + 1) * P, :])
            bq_sb = const.tile([P, 2], f32, tag="bq", name="bq")
            bk_sb = const.tile([P, 2], f32, tag="bk", name="bk")
            nc.sync.dma_start(bq_sb[:], bq2[:])
            nc.sync.dma_start(bk_sb[:], bk2[:])
            bv1_sb = const.tile([1, E1], bf16, tag="bv1", name="bv1s")
            nc.sync.dma_start(bv1_sb[:], bv1[:])
            ones_sb = const.tile([1, P], bf16, tag="ones", name="oness")
            nc.sync.dma_start(ones_sb[:], ones1[:])

            # ---- x (transposed) ----
            xt_sb = [xt_pool.tile([P, S_], bf16, tag=f"xt{d}", name=f"xt{d}") for d in range(2)]
            xq_sb = [xt_pool.tile([P, QC_], bf16, tag=f"xq{d}", name=f"xq{d}") for d in range(2)]
            for d in range(2):
                nc.sync.dma_start(xt_sb[d][:], xT[d * P:(d + 1) * P, :])
                nc.sync.dma_start(xq_sb[d][:], xqT[d * P:(d + 1) * P, :])

            kt_sb = [kqv_pool.tile([P, S_], bf16, tag=f"kt{e}", name=f"kt{e}") for e in range(2)]
            qt_sb = [kqv_pool.tile([P, QC_], bf16, tag=f"qt{e}", name=f"qt{e}") for e in range(2)]
            v_sb = kqv_pool.tile([P, KB * E1], bf16, tag="v", name="vsb")

            for _rep in range(repeats):
                # ---- projections ----
                # KT[e_block] = (Wk.T)^T-slices . xT ; bias via per-partition add
                for eb in range(2):
                    for s0 in range(0, S_, 512):
                        sw = min(512, S_ - s0)
                        ps = ps_c_pool.tile([P, 512], f32, tag="ps_c", name="psc")
                        for d in range(2):
                            nc.tensor.matmul(
                                ps[:, :sw],
                                wk_sb[d][:, eb * P:(eb + 1) * P],
                                xt_sb[d][:, s0:s0 + sw],
                                start=(d == 0), stop=(d == 1),
                            )
                        nc.vector.tensor_scalar_add(
                            kt_sb[eb][:, s0:s0 + sw], ps[:, :sw],
                            bk_sb[:, eb:eb + 1])
                    for q0 in range(0, QC_, 512):
                        qw = min(512, QC_ - q0)
                        ps = ps_c_pool.tile([P, 512], f32, tag="ps_c", name="psc")
                        for d in range(2):
                            nc.tensor.matmul(
                                ps[:, :qw],
                                wq_sb[d][:, eb * P:(eb + 1) * P],
                                xq_sb[d][:, q0:q0 + qw],
                                start=(d == 0), stop=(d == 1),
                            )
                        nc.vector.tensor_scalar_add(
                            qt_sb[eb][:, q0:q0 + qw], ps[:, :qw],
                            bq_sb[:, eb:eb + 1])

                # V[k_block] = xT-slices^T . Wv.T  (+ rank-1 bias & ones col)
                for kb in range(KB):
                    ps = ps_c_pool.tile([P, E1], f32, tag="ps_c", name="psc")
                    for d in range(2):
                        nc.tensor.matmul(
                            ps[:],
                            xt_sb[d][:, kb * P:(kb + 1) * P],
                            wv_sb[d][:],
                            start=(d == 0), stop=False,
                        )
                    nc.tensor.matmul(ps[:], ones_sb[:], bv1_sb[:],
                                     start=False, stop=True)
                    nc.vector.tensor_copy(v_sb[:, kb * E1:(kb + 1) * E1], ps[:])

                # ---- main loop over query windows ----
                for w in range(NW):
                    vt = valid_pool.tile([P, KB * QW], mdt, tag="valid", name="vt")
                    nc.sync.dma_start(vt[:], validb[w, :, :])
                    pt = pt_pool.tile([P, KB * QW], bf16, tag="pt", name="ptt")

                    # scoresT + exp, two k-blocks per psum tile
                    for p2 in range(KB // 2):
                        ps = ps_s_pool.tile([P, 2 * QW], f32, tag="ps_s", name="pss")
                        for i in range(2):
                            kb = 2 * p2 + i
                            for eb in range(2):
                                nc.tensor.matmul(
                                    ps[:, i * QW:(i + 1) * QW],
                                    kt_sb[eb][:, kb * P:(kb + 1) * P],
                                    qt_sb[eb][:, w * QW:(w + 1) * QW],
                                    start=(eb == 0), stop=(eb == 1),
                                )
                        nc.scalar.activation(
                            pt[:, p2 * 2 * QW:(p2 + 1) * 2 * QW], ps[:], Exp)

                    # multiplicative mask
                    for g0 in range(0, KB, MG):
                        sl = slice(g0 * QW, (g0 + MG) * QW)
                        nc.gpsimd.tensor_tensor(pt[:, sl], pt[:, sl], vt[:, sl],
                                                mult)

                    # context + row-sum column, then normalize
                    for qb in range(QB):
                        pc = ps_c_pool.tile([P, E1], f32, tag="ps_c", name="psc")
                        for kb in range(KB):
                            nc.tensor.matmul(
                                pc[:],
                                pt[:, kb * QW + qb * P: kb * QW + (qb + 1) * P],
                                v_sb[:, kb * E1:(kb + 1) * E1],
                                start=(kb == 0), stop=(kb == KB - 1),
                            )
                        rc = misc_pool.tile([P, 1], f32, tag="rc", name="rct")
                        nc.vector.reciprocal(rc[:], pc[:, D_:E1])
                        cs = ctx_pool.tile([P, D_], f32, tag="ctx", name="cst")
                        nc.vector.tensor_scalar_mul(cs[:], pc[:, :D_], rc[:])
                        r0 = w * QW + qb * P
                        nc.sync.dma_start(out[r0:r0 + P, :], cs[:])

    nc.compile()
    return nc


def prep_core_inputs(input_tensor, attention_mask, Wq, bq, Wk, bk, Wv, bv,
                     core, S_=S, QC_=QC, QW=512, scale=None,
                     mask_mode=MASK_MODE):
    """Host-side shard + layout prep for one core. All args are numpy."""
    D_ = D
    KB = S_ // P
    NW = QC_ // QW
    E1 = D_ + 1
    if scale is None:
        scale = float(np.sqrt(np.float32(S_)))
    b, h = core // 2, core % 2
    q0 = h * QC_

    xT = np.ascontiguousarray(input_tensor[b].T).astype(BF16)          # [D,S]
    xqT = np.ascontiguousarray(xT[:, q0:q0 + QC_])
    wqT = (Wq.T / scale).astype(BF16)
    wkT = np.ascontiguousarray(Wk.T).astype(BF16)
    wvT = np.zeros((D_, E1), dtype=BF16)
    wvT[:, :D_] = Wv.T.astype(BF16)
    bq2 = np.ascontiguousarray((bq / scale).reshape(2, P).T).astype(np.float32)
    bk2 = np.ascontiguousarray(bk.reshape(2, P).T).astype(np.float32)
    bv1 = np.concatenate([bv, [1.0]]).reshape(1, E1).astype(BF16)
    ones1 = np.ones((1, P), dtype=BF16)

    valid = (~attention_mask[b, q0:q0 + QC_, :]).T                      # [S,QC]
    vb = valid.reshape(KB, P, NW, QW).transpose(2, 1, 0, 3)
    vb = np.ascontiguousarray(vb.reshape(NW, P, KB * QW))
    if mask_mode == "uint8":
        validb = vb.astype(np.uint8)
    else:
        validb = vb.astype(BF16)

    return {
        "xT": xT, "xqT": xqT, "wqT": wqT, "wkT": wkT, "wvT": wvT,
        "bq2": bq2, "bk2": bk2, "bv1": bv1, "ones1": ones1,
        "validb": validb,
    }


_NC_CACHE = {}


def _get_nc(**kw):
    key = tuple(sorted(kw.items()))
    if key not in _NC_CACHE:
        _NC_CACHE[key] = build_nc(**kw)
    return _NC_CACHE[key]


def kernel(input_tensor, attention_mask, Wq, bq, Wk, bk, Wv, bv):
    input_tensor = np.asarray(input_tensor, dtype=np.float32)
    attention_mask = np.asarray(attention_mask).astype(bool)
    Wq, bq = np.asarray(Wq, np.float32), np.asarray(bq, np.float32)
    Wk, bk = np.asarray(Wk, np.float32), np.asarray(bk, np.float32)
    Wv, bv = np.asarray(Wv, np.float32), np.asarray(bv, np.float32)

    nc = _get_nc()
    in_maps = [
        prep_core_inputs(input_tensor, attention_mask, Wq, bq, Wk, bk, Wv, bv,
                         core=c)
        for c in range(NCORES)
    ]
    res = run_bass_kernel_spmd(nc, in_maps, core_ids=list(range(NCORES)))

    full = np.empty((B, S, D), dtype=np.float32)
    for c in range(NCORES):
        b, h = c // 2, c % 2
        full[b, h * QC:(h + 1) * QC, :] = res.results[c]["out"]
    return full


# revision 8
# speedup vs baseline: 1.1619x; 1.1619x over previous
"""Self-contained Trainium2 Bass kernel for nn_AttentionHead.

Reference computation (per batch b):
    Q = x @ Wq.T + bq ; K = x @ Wk.T + bk ; V = x @ Wv.T + bv
    scores = Q @ K.T / sqrt(S)          (S = 4096, the reference's seq-len quirk)
    scores = where(mask, -1e9, scores)
    ctx = softmax(scores, -1) @ V

Sharding: 8 cores, each takes one (batch, query-half) pair: core c -> batch
c//2, queries [(c%2)*2048, (c%2+1)*2048). K/V are computed per-core from the
full batch input (cheap, avoids collectives entirely).

Device layout (per core):
  - x passed pre-transposed (xT [D,S] bf16) so all matmuls need no on-device
    transposes: projections contract over d with d on partitions.
  - QT/KT [e, s] (e on partitions), V [s, e] natural.
  - scoresT[k, q] = KT.T-slices @ QT  -> softmax probs PT[k, q] with k on
    partitions; L[q] arrives as an extra ones-column appended to V via a
    rank-1 bias matmul, so ctx_psum[q, 0:256] = P@V and ctx_psum[q, 256] = L.
  - mask applied multiplicatively after exp (exp(-1e9) == 0 in the reference
    for every finite row, and rows cannot be fully masked for random masks).
"""

import sys

sys.path.insert(0, "/opt/trn_rl_repo")

import ml_dtypes
import numpy as np

import concourse.bass as bass
import concourse.tile as tile
from concourse import bacc, mybir
from concourse.bass_utils import run_bass_kernel_spmd

BF16 = ml_dtypes.bfloat16

B, S, D = 4, 4096, 256
NCORES = 8
QC = (B * S) // NCORES  # 2048 queries per core
P = 128

# Mask application mode:
#   "fp8"  - mask folded into the scores matmul as a third accumulating
#            matmul (-240*I) @ valid_fp8; exp underflows masked lanes to 0.
#   "uint8" - multiplicative u8 mask on gpsimd after exp.
MASK_MODE = "fp8"
MASK_NEG = -240.0  # exactly representable in fp8e4; exp(-240+s) == 0 in f32


def build_nc(S_=S, QC_=QC, QW=512, repeats=1, mask_mode=MASK_MODE, debug=False):
    """Build the per-core Bass program (same program runs SPMD on all cores)."""
    D_ = D
    KB = S_ // P            # k blocks of 128
    NW = QC_ // QW          # query windows
    QB = QW // P            # 128-row query blocks per window
    MG = min(8, KB)         # kb per mask-multiply op
    E1 = D_ + 1             # V plus ones column
    f32 = mybir.dt.float32
    bf16 = mybir.dt.bfloat16
    mdt = {"uint8": mybir.dt.uint8, "bfloat16": bf16,
           "fp8": mybir.dt.float8e4}[mask_mode]

    nc = bacc.Bacc("TRN2", target_bir_lowering=False, debug=debug,
                   num_devices=NCORES)

    xT = nc.dram_tensor("xT", [D_, S_], bf16, kind="ExternalInput").ap()
    xqT = nc.dram_tensor("xqT", [D_, QC_], bf16, kind="ExternalInput").ap()
    wqT = nc.dram_tensor("wqT", [D_, D_], bf16, kind="ExternalInput").ap()
    wkT = nc.dram_tensor("wkT", [D_, D_], bf16, kind="ExternalInput").ap()
    wvT = nc.dram_tensor("wvT", [D_, E1], bf16, kind="ExternalInput").ap()
    bq2 = nc.dram_tensor("bq2", [P, 2], f32, kind="ExternalInput").ap()
    bk2 = nc.dram_tensor("bk2", [P, 2], f32, kind="ExternalInput").ap()
    bv1 = nc.dram_tensor("bv1", [1, E1], bf16, kind="ExternalInput").ap()
    ones1 = nc.dram_tensor("ones1", [1, P], bf16, kind="ExternalInput").ap()
    validb = nc.dram_tensor("validb", [NW, P, KB * QW], mdt,
                            kind="ExternalInput").ap()
    if mask_mode == "fp8":
        idneg = nc.dram_tensor("idneg", [P, P], mybir.dt.float8e4,
                               kind="ExternalInput").ap()
    out = nc.dram_tensor("out", [QC_, D_], f32, kind="ExternalOutput").ap()

    Exp = mybir.ActivationFunctionType.Exp
    mult = mybir.AluOpType.mult

    with tile.TileContext(nc) as tc:
        with (
            tc.tile_pool(name="const", bufs=1) as const,
            tc.tile_pool(name="xt", bufs=1) as xt_pool,
            tc.tile_pool(name="kqv", bufs=1) as kqv_pool,
            tc.tile_pool(name="valid", bufs=2) as valid_pool,
            tc.tile_pool(name="pt", bufs=2) as pt_pool,
            tc.tile_pool(name="ctx", bufs=3) as ctx_pool,
            tc.tile_pool(name="misc", bufs=4) as misc_pool,
            tc.tile_pool(name="ps_s", bufs=2, space="PSUM") as ps_s_pool,
            tc.tile_pool(name="ps_c", bufs=4, space="PSUM") as ps_c_pool,
        ):
            # ---- constants / weights ----
            wq_sb = [const.tile([P, D_], bf16, tag=f"wq{d}", name=f"wq{d}") for d in range(2)]
            wk_sb = [const.tile([P, D_], bf16, tag=f"wk{d}", name=f"wk{d}") for d in range(2)]
            wv_sb = [const.tile([P, E1], bf16, tag=f"wv{d}", name=f"wv{d}") for d in range(2)]
            for d in range(2):
                nc.sync.dma_start(wq_sb[d][:], wqT[d * P:(d + 1) * P, :])
                nc.sync.dma_start(wk_sb[d][:], wkT[d * P:(d + 1) * P, :])
                nc.sync.dma_start(wv_sb[d][:], wvT[d * P:(d + 1) * P, :])
            bq_sb = const.tile([P, 2], f32, tag="bq", name="bq")
            bk_sb = const.tile([P, 2], f32, tag="bk", name="bk")
            nc.sync.dma_start(bq_sb[:], bq2[:])
            nc.sync.dma_start(bk_sb[:], bk2[:])
            bv1_sb = const.tile([1, E1], bf16, tag="bv1", name="bv1s")
            nc.sync.dma_start(bv1_sb[:], bv1[:])
            ones_sb = const.tile([1, P], bf16, tag="ones", name="oness")
            nc.sync.dma_start(ones_sb[:], ones1[:])
            if mask_mode == "fp8":
                idneg_sb = const.tile([P, P], mybir.dt.float8e4, tag="idneg",
                                      name="idnegs")
                nc.sync.dma_start(idneg_sb[:], idneg[:])

            # ---- x (transposed) ----
            xt_sb = [xt_pool.tile([P, S_], bf16, tag=f"xt{d}", name=f"xt{d}") for d in range(2)]
            xq_sb = [xt_pool.tile([P, QC_], bf16, tag=f"xq{d}", name=f"xq{d}") for d in range(2)]
            for d in range(2):
                nc.sync.dma_start(xt_sb[d][:], xT[d * P:(d + 1) * P, :])
                nc.sync.dma_start(xq_sb[d][:], xqT[d * P:(d + 1) * P, :])

            kt_sb = [kqv_pool.tile([P, S_], bf16, tag=f"kt{e}", name=f"kt{e}") for e in range(2)]
            qt_sb = [kqv_pool.tile([P, QC_], bf16, tag=f"qt{e}", name=f"qt{e}") for e in range(2)]
            v_sb = kqv_pool.tile([P, KB * E1], bf16, tag="v", name="vsb")

            for _rep in range(repeats):
                # ---- projections ----
                # KT[e_block] = (Wk.T)^T-slices . xT ; bias via per-partition add
                for eb in range(2):
                    for s0 in range(0, S_, 512):
                        sw = min(512, S_ - s0)
                        ps = ps_c_pool.tile([P, 512], f32, tag="ps_c", name="psc")
                        for d in range(2):
                            nc.tensor.matmul(
                                ps[:, :sw],
                                wk_sb[d][:, eb * P:(eb + 1) * P],
                                xt_sb[d][:, s0:s0 + sw],
                                start=(d == 0), stop=(d == 1),
                            )
                        nc.vector.tensor_scalar_add(
                            kt_sb[eb][:, s0:s0 + sw], ps[:, :sw],
                            bk_sb[:, eb:eb + 1])
                    for q0 in range(0, QC_, 512):
                        qw = min(512, QC_ - q0)
                        ps = ps_c_pool.tile([P, 512], f32, tag="ps_c", name="psc")
                        for d in range(2):
                            nc.tensor.matmul(
                                ps[:, :qw],
                                wq_sb[d][:, eb * P:(eb + 1) * P],
                                xq_sb[d][:, q0:q0 + qw],
                                start=(d == 0), stop=(d == 1),
                            )
                        nc.vector.tensor_scalar_add(
                            qt_sb[eb][:, q0:q0 + qw], ps[:, :qw],
                            bq_sb[:, eb:eb + 1])

                # V[k_block] = xT-slices^T . Wv.T  (+ rank-1 bias & ones col)
                for kb in range(KB):
                    ps = ps_c_pool.tile([P, E1], f32, tag="ps_c", name="psc")
                    for d in range(2):
                        nc.tensor.matmul(
                            ps[:],
                            xt_sb[d][:, kb * P:(kb + 1) * P],
                            wv_sb[d][:],
                            start=(d == 0), stop=False,
                        )
                    nc.tensor.matmul(ps[:], ones_sb[:], bv1_sb[:],
                                     start=False, stop=True)
                    nc.vector.tensor_copy(v_sb[:, kb * E1:(kb + 1) * E1], ps[:])

                # ---- main loop over query windows ----
                for w in range(NW):
                    vt = valid_pool.tile([P, KB * QW], mdt, tag="valid", name="vt")
                    nc.sync.dma_start(vt[:], validb[w, :, :])
                    pt = pt_pool.tile([P, KB * QW], bf16, tag="pt", name="ptt")

                    # scoresT (+ additive fp8 mask) + exp, two k-blocks per
                    # psum tile
                    for p2 in range(KB // 2):
                        ps = ps_s_pool.tile([P, 2 * QW], f32, tag="ps_s", name="pss")
                        for i in range(2):
                            kb = 2 * p2 + i
                            for eb in range(2):
                                nc.tensor.matmul(
                                    ps[:, i * QW:(i + 1) * QW],
                                    kt_sb[eb][:, kb * P:(kb + 1) * P],
                                    qt_sb[eb][:, w * QW:(w + 1) * QW],
                                    start=(eb == 0),
                                    stop=(eb == 1 and mask_mode != "fp8"),
                                )
                            if mask_mode == "fp8":
                                nc.tensor.matmul(
                                    ps[:, i * QW:(i + 1) * QW],
                                    idneg_sb[:],
                                    vt[:, kb * QW:(kb + 1) * QW],
                                    start=False, stop=True,
                                )
                        nc.scalar.activation(
                            pt[:, p2 * 2 * QW:(p2 + 1) * 2 * QW], ps[:], Exp)

                    if mask_mode != "fp8":
                        # multiplicative mask
                        for g0 in range(0, KB, MG):
                            sl = slice(g0 * QW, (g0 + MG) * QW)
                            nc.gpsimd.tensor_tensor(pt[:, sl], pt[:, sl],
                                                    vt[:, sl], mult)

                    # context + row-sum column, then normalize
                    for qb in range(QB):
                        pc = ps_c_pool.tile([P, E1], f32, tag="ps_c", name="psc")
                        for kb in range(KB):
                            nc.tensor.matmul(
                                pc[:],
                                pt[:, kb * QW + qb * P: kb * QW + (qb + 1) * P],
                                v_sb[:, kb * E1:(kb + 1) * E1],
                                start=(kb == 0), stop=(kb == KB - 1),
                            )
                        rc = misc_pool.tile([P, 1], f32, tag="rc", name="rct")
                        nc.vector.reciprocal(rc[:], pc[:, D_:E1])
                        cs = ctx_pool.tile([P, D_], f32, tag="ctx", name="cst")
                        nc.vector.tensor_scalar_mul(cs[:], pc[:, :D_], rc[:])
                        r0 = w * QW + qb * P
                        nc.sync.dma_start(out[r0:r0 + P, :], cs[:])

    nc.compile()
    return nc


def prep_core_inputs(input_tensor, attention_mask, Wq, bq, Wk, bk, Wv, bv,
                     core, S_=S, QC_=QC, QW=512, scale=None,
                     mask_mode=MASK_MODE):
    """Host-side shard + layout prep for one core. All args are numpy."""
    D_ = D
    KB = S_ // P
    NW = QC_ // QW
    E1 = D_ + 1
    if scale is None:
        scale = float(np.sqrt(np.float32(S_)))
    b, h = core // 2, core % 2
    q0 = h * QC_

    xT = np.ascontiguousarray(input_tensor[b].T).astype(BF16)          # [D,S]
    xqT = np.ascontiguousarray(xT[:, q0:q0 + QC_])
    wqT = (Wq.T / scale).astype(BF16)
    wkT = np.ascontiguousarray(Wk.T).astype(BF16)
    wvT = np.zeros((D_, E1), dtype=BF16)
    wvT[:, :D_] = Wv.T.astype(BF16)
    bq2 = np.ascontiguousarray((bq / scale).reshape(2, P).T).astype(np.float32)
    bk2 = np.ascontiguousarray(bk.reshape(2, P).T).astype(np.float32)
    bv1 = np.concatenate([bv, [1.0]]).reshape(1, E1).astype(BF16)
    ones1 = np.ones((1, P), dtype=BF16)

    if mask_mode == "fp8":
        mk = attention_mask[b, q0:q0 + QC_, :].T                        # [S,QC]
    else:
        mk = ~attention_mask[b, q0:q0 + QC_, :].T
    vb = mk.reshape(KB, P, NW, QW).transpose(2, 1, 0, 3)
    vb = np.ascontiguousarray(vb.reshape(NW, P, KB * QW))
    if mask_mode == "uint8":
        validb = vb.astype(np.uint8)
    elif mask_mode == "fp8":
        validb = vb.astype(ml_dtypes.float8_e4m3)
    else:
        validb = vb.astype(BF16)

    ret = {
        "xT": xT, "xqT": xqT, "wqT": wqT, "wkT": wkT, "wvT": wvT,
        "bq2": bq2, "bk2": bk2, "bv1": bv1, "ones1": ones1,
        "validb": validb,
    }
    if mask_mode == "fp8":
        ret["idneg"] = (MASK_NEG * np.eye(P)).astype(ml_dtypes.float8_e4m3)
    return ret


_NC_CACHE = {}


def _get_nc(**kw):
    key = tuple(sorted(kw.items()))
    if key not in _NC_CACHE:
        _NC_CACHE[key] = build_nc(**kw)
    return _NC_CACHE[key]


def kernel(input_tensor, attention_mask, Wq, bq, Wk, bk, Wv, bv):
    input_tensor = np.asarray(input_tensor, dtype=np.float32)
    attention_mask = np.asarray(attention_mask).astype(bool)
    Wq, bq = np.asarray(Wq, np.float32), np.asarray(bq, np.float32)
    Wk, bk = np.asarray(Wk, np.float32), np.asarray(bk, np.float32)
    Wv, bv = np.asarray(Wv, np.float32), np.asarray(bv, np.float32)

    nc = _get_nc()
    in_maps = [
        prep_core_inputs(input_tensor, attention_mask, Wq, bq, Wk, bk, Wv, bv,
                         core=c)
        for c in range(NCORES)
    ]
    res = run_bass_kernel_spmd(nc, in_maps, core_ids=list(range(NCORES)))

    full = np.empty((B, S, D), dtype=np.float32)
    for c in range(NCORES):
        b, h = c // 2, c % 2
        full[b, h * QC:(h + 1) * QC, :] = res.results[c]["out"]
    return full


# revision 15
# speedup vs baseline: 1.2853x; 1.1061x over previous
"""Self-contained Trainium2 Bass kernel for nn_AttentionHead.

Reference computation (per batch b):
    Q = x @ Wq.T + bq ; K = x @ Wk.T + bk ; V = x @ Wv.T + bv
    scores = Q @ K.T / sqrt(S)          (S = 4096, the reference's seq-len quirk)
    scores = where(mask, -1e9, scores)
    ctx = softmax(scores, -1) @ V

Sharding: 8 cores, each takes one (batch, query-half) pair: core c -> batch
c//2, queries [(c%2)*2048, (c%2+1)*2048). K/V are computed per-core from the
full batch input (cheap, avoids collectives entirely).

Device layout (per core):
  - x passed pre-transposed (xT [D,S] bf16) so all matmuls need no on-device
    transposes: projections contract over d with d on partitions.
  - QT/KT [e, s] (e on partitions), V [s, e] natural.
  - scoresT[k, q] = KT.T-slices @ QT  -> softmax probs PT[k, q] with k on
    partitions; L[q] arrives as an extra ones-column appended to V via a
    rank-1 bias matmul, so ctx_psum[q, 0:256] = P@V and ctx_psum[q, 256] = L.
  - mask applied multiplicatively after exp (exp(-1e9) == 0 in the reference
    for every finite row, and rows cannot be fully masked for random masks).
"""

import sys

sys.path.insert(0, "/opt/trn_rl_repo")

import ml_dtypes
import numpy as np

import concourse.bass as bass
import concourse.tile as tile
from concourse import bacc, mybir
from concourse.bass_utils import run_bass_kernel_spmd

BF16 = ml_dtypes.bfloat16

B, S, D = 4, 4096, 256
NCORES = 8
QC = (B * S) // NCORES  # 2048 queries per core
P = 128

# Mask application mode:
#   "fp8"  - mask folded into the scores matmul as a third accumulating
#            matmul (-240*I) @ valid_fp8; exp underflows masked lanes to 0.
#   "uint8" - multiplicative u8 mask on gpsimd after exp.
MASK_MODE = "fp8"
# Mask constants: contribution is MASK_NEG * MASK_VAL = -7680 raw, which
# after the exp's 1/sqrt(S) affine scale (>= 1/64 here) is <= -120 -> exp
# underflows to exactly 0.0 in f32. Both factors are exactly representable
# in fp8e4 (this ml_dtypes variant's max finite is 240).
MASK_NEG = -240.0
MASK_VAL = 32.0
SCORES_FP8 = True  # QK^T via one fp8 DoubleRow matmul (contraction 256)


def build_nc(S_=S, QC_=QC, QW=512, repeats=1, mask_mode=MASK_MODE,
             scores_fp8=SCORES_FP8, debug=False):
    """Build the per-core Bass program (same program runs SPMD on all cores)."""
    D_ = D
    KB = S_ // P            # k blocks of 128
    NW = QC_ // QW          # query windows
    QB = QW // P            # 128-row query blocks per window
    MG = min(8, KB)         # kb per mask-multiply op
    E1 = D_ + 1             # V plus ones column
    f32 = mybir.dt.float32
    bf16 = mybir.dt.bfloat16
    fp8 = mybir.dt.float8e4
    mdt = {"uint8": mybir.dt.uint8, "bfloat16": bf16,
           "fp8": fp8}[mask_mode]
    if scores_fp8:
        assert mask_mode == "fp8" and QW <= 512
    inv_scale = float(1.0 / np.sqrt(np.float32(S_)))

    nc = bacc.Bacc("TRN2", target_bir_lowering=False, debug=debug,
                   num_devices=NCORES)

    xT = nc.dram_tensor("xT", [D_, S_], bf16, kind="ExternalInput").ap()
    xqT = nc.dram_tensor("xqT", [D_, QC_], bf16, kind="ExternalInput").ap()
    wqT = nc.dram_tensor("wqT", [D_, D_], bf16, kind="ExternalInput").ap()
    wkT = nc.dram_tensor("wkT", [D_, D_], bf16, kind="ExternalInput").ap()
    wvT = nc.dram_tensor("wvT", [D_, E1], bf16, kind="ExternalInput").ap()
    bq2 = nc.dram_tensor("bq2", [P, 2], f32, kind="ExternalInput").ap()
    bk2 = nc.dram_tensor("bk2", [P, 2], f32, kind="ExternalInput").ap()
    bv1 = nc.dram_tensor("bv1", [1, E1], bf16, kind="ExternalInput").ap()
    ones1 = nc.dram_tensor("ones1", [1, P], bf16, kind="ExternalInput").ap()
    validb = nc.dram_tensor("validb", [NW, P, KB * QW], mdt,
                            kind="ExternalInput").ap()
    if mask_mode == "fp8":
        idneg = nc.dram_tensor("idneg", [P, P], mybir.dt.float8e4,
                               kind="ExternalInput").ap()
    out = nc.dram_tensor("out", [QC_, D_], f32, kind="ExternalOutput").ap()

    Exp = mybir.ActivationFunctionType.Exp
    mult = mybir.AluOpType.mult

    with tile.TileContext(nc) as tc:
        with (
            tc.tile_pool(name="const", bufs=1) as const,
            tc.tile_pool(name="xt", bufs=1) as xt_pool,
            tc.tile_pool(name="kqv", bufs=1) as kqv_pool,
            tc.tile_pool(name="valid", bufs=2) as valid_pool,
            tc.tile_pool(name="pt", bufs=2) as pt_pool,
            tc.tile_pool(name="ctx", bufs=3) as ctx_pool,
            tc.tile_pool(name="misc", bufs=4) as misc_pool,
            tc.tile_pool(name="ps_s", bufs=2, space="PSUM") as ps_s_pool,
            tc.tile_pool(name="ps_c", bufs=4, space="PSUM") as ps_c_pool,
        ):
            # ---- constants / weights ----
            wq_sb = [const.tile([P, D_], bf16, tag=f"wq{d}", name=f"wq{d}") for d in range(2)]
            wk_sb = [const.tile([P, D_], bf16, tag=f"wk{d}", name=f"wk{d}") for d in range(2)]
            wv_sb = [const.tile([P, E1], bf16, tag=f"wv{d}", name=f"wv{d}") for d in range(2)]
            for d in range(2):
                nc.sync.dma_start(wq_sb[d][:], wqT[d * P:(d + 1) * P, :])
                nc.sync.dma_start(wk_sb[d][:], wkT[d * P:(d + 1) * P, :])
                nc.sync.dma_start(wv_sb[d][:], wvT[d * P:(d + 1) * P, :])
            bq_sb = const.tile([P, 2], f32, tag="bq", name="bq")
            bk_sb = const.tile([P, 2], f32, tag="bk", name="bk")
            nc.sync.dma_start(bq_sb[:], bq2[:])
            nc.sync.dma_start(bk_sb[:], bk2[:])
            bv1_sb = const.tile([1, E1], bf16, tag="bv1", name="bv1s")
            nc.sync.dma_start(bv1_sb[:], bv1[:])
            ones_sb = const.tile([1, P], bf16, tag="ones", name="oness")
            nc.sync.dma_start(ones_sb[:], ones1[:])
            if mask_mode == "fp8":
                idneg_sb = const.tile([P, P], mybir.dt.float8e4, tag="idneg",
                                      name="idnegs")
                nc.sync.dma_start(idneg_sb[:], idneg[:])

            # ---- x (transposed) ----
            xt_sb = [xt_pool.tile([P, S_], bf16, tag=f"xt{d}", name=f"xt{d}") for d in range(2)]
            xq_sb = [xt_pool.tile([P, QC_], bf16, tag=f"xq{d}", name=f"xq{d}") for d in range(2)]
            for d in range(2):
                nc.sync.dma_start(xt_sb[d][:], xT[d * P:(d + 1) * P, :])
                nc.sync.dma_start(xq_sb[d][:], xqT[d * P:(d + 1) * P, :])

            if scores_fp8:
                # interleaved fp8 layouts for DoubleRow:
                #   kt8[p, kb*256 + j*128 + m] = K^T[e=j*128+p, k=kb*128+m]
                #   qt8[p, w*2*QW + j*QW + q]  = Q^T[e=j*128+p, q=w*QW+q]
                kt8 = kqv_pool.tile([P, KB * 2 * P], fp8, tag="kt8", name="kt8")
                qt8 = kqv_pool.tile([P, 2 * QC_], fp8, tag="qt8", name="qt8")
                kt8v = kt8[:].rearrange("p (kb j m) -> p kb j m", j=2, m=P)
            else:
                kt_sb = [kqv_pool.tile([P, S_], bf16, tag=f"kt{e}", name=f"kt{e}") for e in range(2)]
                qt_sb = [kqv_pool.tile([P, QC_], bf16, tag=f"qt{e}", name=f"qt{e}") for e in range(2)]
            v_sb = kqv_pool.tile([P, KB * E1], bf16, tag="v", name="vsb")

            for _rep in range(repeats):
                # ---- projections ----
                # KT[e_block] = (Wk.T)^T-slices . xT ; bias via per-partition add
                for eb in range(2):
                    for s0 in range(0, S_, 512):
                        sw = min(512, S_ - s0)
                        nkb = sw // P
                        ps = ps_c_pool.tile([P, 512], f32, tag="ps_c", name="psc")
                        for d in range(2):
                            nc.tensor.matmul(
                                ps[:, :sw],
                                wk_sb[d][:, eb * P:(eb + 1) * P],
                                xt_sb[d][:, s0:s0 + sw],
                                start=(d == 0), stop=(d == 1),
                            )
                        if scores_fp8:
                            dst = kt8v[:, s0 // P:s0 // P + nkb, eb, :]
                            src = ps[:, :sw].rearrange("p (kb m) -> p kb m", m=P)
                        else:
                            dst = kt_sb[eb][:, s0:s0 + sw]
                            src = ps[:, :sw]
                        nc.vector.tensor_scalar_add(dst, src,
                                                    bk_sb[:, eb:eb + 1])
                    qstep = QW if scores_fp8 else 512
                    for q0 in range(0, QC_, qstep):
                        qw = min(qstep, QC_ - q0)
                        ps = ps_c_pool.tile([P, 512], f32, tag="ps_c", name="psc")
                        for d in range(2):
                            nc.tensor.matmul(
                                ps[:, :qw],
                                wq_sb[d][:, eb * P:(eb + 1) * P],
                                xq_sb[d][:, q0:q0 + qw],
                                start=(d == 0), stop=(d == 1),
                            )
                        if scores_fp8:
                            dst = qt8[:, q0 * 2 + eb * QW:
                                      q0 * 2 + eb * QW + qw]
                        else:
                            dst = qt_sb[eb][:, q0:q0 + qw]
                        nc.vector.tensor_scalar_add(dst, ps[:, :qw],
                                                    bq_sb[:, eb:eb + 1])

                # V[k_block] = xT-slices^T . Wv.T  (+ rank-1 bias & ones col)
                for kb in range(KB):
                    ps = ps_c_pool.tile([P, E1], f32, tag="ps_c", name="psc")
                    for d in range(2):
                        nc.tensor.matmul(
                            ps[:],
                            xt_sb[d][:, kb * P:(kb + 1) * P],
                            wv_sb[d][:],
                            start=(d == 0), stop=False,
                        )
                    nc.tensor.matmul(ps[:], ones_sb[:], bv1_sb[:],
                                     start=False, stop=True)
                    nc.vector.tensor_copy(v_sb[:, kb * E1:(kb + 1) * E1], ps[:])

                # ---- main loop over query windows ----
                for w in range(NW):
                    vt = valid_pool.tile([P, KB * QW], mdt, tag="valid", name="vt")
                    nc.sync.dma_start(vt[:], validb[w, :, :])
                    pt = pt_pool.tile([P, KB * QW], bf16, tag="pt", name="ptt")

                    # scoresT (+ additive fp8 mask) + exp, two k-blocks per
                    # psum tile; 1/sqrt(S) folded into the exp affine
                    if scores_fp8:
                        qt8w = qt8[:, w * 2 * QW:(w + 1) * 2 * QW].rearrange(
                            "p (j q) -> p j q", j=2)
                    for p2 in range(KB // 2):
                        ps = ps_s_pool.tile([P, 2 * QW], f32, tag="ps_s", name="pss")
                        for i in range(2):
                            kb = 2 * p2 + i
                            if scores_fp8:
                                nc.tensor.matmul(
                                    ps[:, i * QW:(i + 1) * QW],
                                    kt8v[:, kb, :, :],
                                    qt8w,
                                    start=True, stop=False,
                                    perf_mode=mybir.MatmulPerfMode.DoubleRow,
                                )
                            else:
                                for eb in range(2):
                                    nc.tensor.matmul(
                                        ps[:, i * QW:(i + 1) * QW],
                                        kt_sb[eb][:, kb * P:(kb + 1) * P],
                                        qt_sb[eb][:, w * QW:(w + 1) * QW],
                                        start=(eb == 0),
                                        stop=(eb == 1 and mask_mode != "fp8"),
                                    )
                            if mask_mode == "fp8":
                                nc.tensor.matmul(
                                    ps[:, i * QW:(i + 1) * QW],
                                    idneg_sb[:],
                                    vt[:, kb * QW:(kb + 1) * QW],
                                    start=False, stop=True,
                                )
                        nc.scalar.activation(
                            pt[:, p2 * 2 * QW:(p2 + 1) * 2 * QW], ps[:], Exp,
                            scale=inv_scale)

                    if mask_mode != "fp8":
                        # multiplicative mask
                        for g0 in range(0, KB, MG):
                            sl = slice(g0 * QW, (g0 + MG) * QW)
                            nc.gpsimd.tensor_tensor(pt[:, sl], pt[:, sl],
                                                    vt[:, sl], mult)

                    # context + row-sum column, then normalize
                    for qb in range(QB):
                        pc = ps_c_pool.tile([P, E1], f32, tag="ps_c", name="psc")
                        for kb in range(KB):
                            nc.tensor.matmul(
                                pc[:],
                                pt[:, kb * QW + qb * P: kb * QW + (qb + 1) * P],
                                v_sb[:, kb * E1:(kb + 1) * E1],
                                start=(kb == 0), stop=(kb == KB - 1),
                            )
                        rc = misc_pool.tile([P, 1], f32, tag="rc", name="rct")
                        nc.vector.reciprocal(rc[:], pc[:, D_:E1])
                        cs = ctx_pool.tile([P, D_], f32, tag="ctx", name="cst")
                        nc.vector.tensor_scalar_mul(cs[:], pc[:, :D_], rc[:])
                        r0 = w * QW + qb * P
                        nc.sync.dma_start(out[r0:r0 + P, :], cs[:])

    nc.compile()
    return nc


def prep_core_inputs(input_tensor, attention_mask, Wq, bq, Wk, bk, Wv, bv,
                     core, S_=S, QC_=QC, QW=512, scale=None,
                     mask_mode=MASK_MODE):
    """Host-side shard + layout prep for one core. All args are numpy."""
    D_ = D
    KB = S_ // P
    NW = QC_ // QW
    E1 = D_ + 1
    if scale is None:
        scale = float(np.sqrt(np.float32(S_)))
    b, h = core // 2, core % 2
    q0 = h * QC_

    xT = np.ascontiguousarray(input_tensor[b].T).astype(BF16)          # [D,S]
    xqT = np.ascontiguousarray(xT[:, q0:q0 + QC_])
    wqT = Wq.T.astype(BF16)  # 1/sqrt(S) is folded into the exp affine
    wkT = np.ascontiguousarray(Wk.T).astype(BF16)
    wvT = np.zeros((D_, E1), dtype=BF16)
    wvT[:, :D_] = Wv.T.astype(BF16)
    bq2 = np.ascontiguousarray(bq.reshape(2, P).T).astype(np.float32)
    bk2 = np.ascontiguousarray(bk.reshape(2, P).T).astype(np.float32)
    bv1 = np.concatenate([bv, [1.0]]).reshape(1, E1).astype(BF16)
    ones1 = np.ones((1, P), dtype=BF16)

    if mask_mode == "fp8":
        mk = attention_mask[b, q0:q0 + QC_, :].T                        # [S,QC]
    else:
        mk = ~attention_mask[b, q0:q0 + QC_, :].T
    vb = mk.reshape(KB, P, NW, QW).transpose(2, 1, 0, 3)
    vb = np.ascontiguousarray(vb.reshape(NW, P, KB * QW))
    if mask_mode == "uint8":
        validb = vb.astype(np.uint8)
    elif mask_mode == "fp8":
        validb = (vb.astype(np.float32) * MASK_VAL).astype(ml_dtypes.float8_e4m3)
    else:
        validb = vb.astype(BF16)

    ret = {
        "xT": xT, "xqT": xqT, "wqT": wqT, "wkT": wkT, "wvT": wvT,
        "bq2": bq2, "bk2": bk2, "bv1": bv1, "ones1": ones1,
        "validb": validb,
    }
    if mask_mode == "fp8":
        ret["idneg"] = (MASK_NEG * np.eye(P)).astype(ml_dtypes.float8_e4m3)
    return ret


_NC_CACHE = {}


def _get_nc(**kw):
    key = tuple(sorted(kw.items()))
    if key not in _NC_CACHE:
        _NC_CACHE[key] = build_nc(**kw)
    return _NC_CACHE[key]


def kernel(input_tensor, attention_mask, Wq, bq, Wk, bk, Wv, bv):
    input_tensor = np.asarray(input_tensor, dtype=np.float32)
    attention_mask = np.asarray(attention_mask).astype(bool)
    Wq, bq = np.asarray(Wq, np.float32), np.asarray(bq, np.float32)
    Wk, bk = np.asarray(Wk, np.float32), np.asarray(bk, np.float32)
    Wv, bv = np.asarray(Wv, np.float32), np.asarray(bv, np.float32)

    nc = _get_nc()
    in_maps = [
        prep_core_inputs(input_tensor, attention_mask, Wq, bq, Wk, bk, Wv, bv,
                         core=c)
        for c in range(NCORES)
    ]
    res = run_bass_kernel_spmd(nc, in_maps, core_ids=list(range(NCORES)))

    full = np.empty((B, S, D), dtype=np.float32)
    for c in range(NCORES):
        b, h = c // 2, c % 2
        full[b, h * QC:(h + 1) * QC, :] = res.results[c]["out"]
    return full
